# revision 29
# baseline (speedup 1.0000x reference)
"""Trainium2 Bass kernel for BinaryDecorator:
    out = (sign(x) @ sign(W).T + b) * mean(|x|)

x: [524288, 128] fp32, W: [128, 128] fp32, b: [128] fp32.

Strategy (8 NeuronCores, data-parallel over rows of x):
  Phase 1 (per core, 65536 rows): stream x from HBM once. For each
    128x128 tile: PE-transpose it to PSUM, take Sign on ScalarE while
    copying PSUM -> SBUF as fp8 (+-1 exact) into a persistent 8MB SBUF
    stash laid out [k=128 partitions, n free] (matmul-ready). VectorE
    accumulates per-partition sums of |x| on the natural-layout tile.
  Phase 2 (global mean, minimal-latency chain): per-partition partials
    -> PE ones-matmul cross-partition reduce folded with the 1/(N*D)
    scale -> a single [1,1] local mean -> AllGather (cheaper than
    AllReduce: no x1.875 reduce pricing) -> [1,8] -> vector sum ->
    PE broadcast of the scalar to all 128 partitions. The bias
    broadcast [128, 512] is prebuilt UNSCALED off the critical path
    (PE ones-matmul) and scaled by the mean with one vector op.
  Phase 3: for each row tile, matmul(lhsT=stash slice fp8, rhs=sign(W).T
    fp8) -> PSUM [n,128] (exact integer counts), then one fused
    VectorE scalar_tensor_tensor (psum*mean + bias*mean), DMA out.

HBM traffic per core = 32MB read + 32MB write = the roofline minimum;
the read and write phases are inherently serialized by the global-mean
dependency, so the critical path is read-stream + mean-chain +
write-stream.

A hand-rolled remote-DMA peer exchange (coll_mode="rdma") is also
implemented and CORRECT: 7 single-dest relative remote_dma_broadcasts
with XOR-slot addressing exchange the [128,1] partials peer-to-peer,
with the arrival wait injected onto the reducing instruction AFTER
tile scheduling (the single-core scheduling sim cannot observe remote
sem increments and would report a deadlock). It measures ~11us SLOWER
than the AllGather here (7 serial SWDGE desc-gens + per-arrival sem
propagation outweigh the collective's constant in this backend), so
allgather stays the default.
"""

import sys

for _p in ("/opt/trn_rl_repo",):
    if _p not in sys.path:
        sys.path.append(_p)

import numpy as np

import concourse.bass as bass
import concourse.mybir as mybir
import concourse.tile as tile
from concourse import bacc, bass_isa, bass_utils
from concourse.bass import ds
from concourse.masks import make_identity
from concourse.tile import add_dep_helper

N_TOTAL = 524288
D = 128
NCORES = 8
N_PER_CORE = N_TOTAL // NCORES
P = 128
T_SUB = 8  # 128-row subtiles per iteration (1024 rows / 512KB per DMA)
BANK = 512  # one full PSUM bank (512 fp32/partition); elementwise op granularity

F32 = mybir.dt.float32
FP8 = mybir.dt.float8e4
AF = mybir.ActivationFunctionType


def make_pools(tc, ctx):
    return dict(
        const=ctx.enter_context(tc.tile_pool(name="const", bufs=1)),
        stash=ctx.enter_context(tc.tile_pool(name="stash", bufs=1)),
        xin=ctx.enter_context(tc.tile_pool(name="xin", bufs=6)),
        outp=ctx.enter_context(tc.tile_pool(name="outp", bufs=6)),
        ptp=ctx.enter_context(tc.tile_pool(name="ptp", bufs=4, space="PSUM")),
        pmm=ctx.enter_context(tc.tile_pool(name="pmm", bufs=4, space="PSUM")),
        dram=ctx.enter_context(tc.tile_pool(name="dram", bufs=1, space="DRAM")),
    )


def emit(
    tc,
    pools,
    out_ap,
    x_ap,
    w_ap,
    b_ap,
    total_elems,
    ncores,
    coll_mode="allgather",
    rdma_sems=None,
    rdma_probe_delay=0,
):
    nc = tc.nc
    n_rows = x_ap.shape[0]
    rows_per_iter = T_SUB * P
    assert n_rows % rows_per_iter == 0
    iters = n_rows // rows_per_iter

    # row->partition permutation chosen so each partition's slice of one
    # iteration is CONTIGUOUS in DRAM (T_SUB rows x 512B = 4KB bursts), and
    # applied identically to input and output so every row lands correctly.
    n_it = n_rows // (T_SUB * P)
    x_view = x_ap.rearrange("(p i t) k -> i p t k", p=P, i=n_it, t=T_SUB)
    out_view = out_ap.rearrange("(p i t) k -> i p t k", p=P, i=n_it, t=T_SUB)

    if True:
        const = pools["const"]
        stash = pools["stash"]
        xin = pools["xin"]
        outp = pools["outp"]
        ptp = pools["ptp"]
        pmm = pools["pmm"]
        dram = pools["dram"]

        identity = const.tile([P, P], F32, name="identity")
        make_identity(nc, identity)

        # --- weights: sign(W)^T as fp8, laid out [k, o] ---
        w_nat = const.tile([P, P], F32, name="w_nat")
        nc.sync.dma_start(w_nat[:], w_ap)
        psum_w = ptp.tile([P, BANK], F32, name="tp", tag="tp")
        nc.tensor.transpose(psum_w[:, :P], w_nat[:], identity[:])
        wsT = const.tile([P, P], FP8, name="wsT")
        nc.scalar.activation(wsT[:], psum_w[:, :P], AF.Sign)

        # bias replicated 4x along free (for the K=1 bias matmul per bank)
        bias4_row = const.tile([1, BANK], F32, name="bias4_row")
        for q in range(BANK // D):
            nc.sync.dma_start(bias4_row[:, ds(q * D, D)], b_ap[None, :])

        xbT = stash.tile([P, n_rows], FP8, name="xbT")
        acc_all = const.tile([P, iters], F32, name="acc_all")

        # --- phase 1: stream x, stash sign(x)^T, accumulate |x| ---
        x_load_insts = []
        banks_per_iter = (T_SUB * P) // BANK
        t_per_bank = BANK // P
        for i in range(iters):
            x_nat = xin.tile([P, T_SUB, P], F32, name="x_nat", tag="x_nat")
            x_load_insts.append(nc.sync.dma_start(x_nat[:], x_view[i]))
            for b in range(banks_per_iter):
                bank = ptp.tile([P, BANK], F32, name="tp", tag="tp")
                for t in range(t_per_bank):
                    nc.tensor.transpose(
                        bank[:, ds(t * P, P)],
                        x_nat[:, b * t_per_bank + t, :],
                        identity[:],
                    )
                col = i * T_SUB * P + b * BANK
                nc.scalar.activation(xbT[:, ds(col, BANK)], bank[:], AF.Sign)
            # |x| row-sums on DVE (walrus rejects accum_out on tensor_scalar)
            nc.vector.tensor_reduce(
                acc_all[:, i : i + 1],
                x_nat[:],
                axis=mybir.AxisListType.XY,
                op=mybir.AluOpType.add,
                apply_absolute_value=True,
            )

        # --- phase 2: global mean(|x|) ---
        # Constants prepared off the critical path: 1/T column for the PE
        # cross-partition reduce, a ones row for PE broadcasts, and the
        # UNSCALED bias broadcast [P, BANK] (scaled by mean after the
        # collective with one vector op).
        inv_col = const.tile([P, 1], F32, name="inv_col")
        nc.vector.memset(inv_col[:], 1.0 / float(total_elems))
        ones_row = const.tile([1, P], F32, name="ones_row")
        nc.vector.memset(ones_row[:], 1.0)
        psum_bb = ptp.tile([P, BANK], F32, name="tp", tag="tp")
        nc.tensor.matmul(
            psum_bb[:], ones_row[:], bias4_row[:], start=True, stop=True
        )
        bias_bb_u = const.tile([P, BANK], F32, name="bias_bb_u")
        nc.scalar.copy(bias_bb_u[:], psum_bb[:])

        acc_col = const.tile([P, 1], F32, name="acc_col")
        nc.vector.tensor_reduce(
            acc_col[:],
            acc_all[:],
            axis=mybir.AxisListType.X,
            op=mybir.AluOpType.add,
        )
        if coll_mode == "allreduce":
            cc_in = dram.tile([P, 1], F32, name="cc_in")
            cc_out = dram.tile([P, 1], F32, name="cc_out", addr_space="Shared")
            nc.sync.dma_start(cc_in[:], acc_col[:])
            nc.gpsimd.collective_compute(
                "AllReduce",
                mybir.AluOpType.add,
                replica_groups=[list(range(ncores))],
                ins=[cc_in[:].opt()],
                outs=[cc_out[:].opt()],
            )
            allred = const.tile([P, 1], F32, name="allred")
            nc.sync.dma_start(allred[:], cc_out[:])
            tot = const.tile([P, 1], F32, name="tot")
            nc.gpsimd.partition_all_reduce(
                tot[:], allred[:], channels=P, reduce_op=bass_isa.ReduceOp.add
            )
            mean_col = const.tile([P, 1], F32, name="mean_col")
            nc.scalar.mul(mean_col[:], tot[:], 1.0 / float(total_elems))
        else:
            gmean = const.tile([1, 1], F32, name="gmean")
            if coll_mode == "rdma":
                # Peer exchange of the [P,1] per-partition partials via
                # remote-DMA broadcasts with XOR-peer addressing: for slot j,
                # every core sends acc_col to peer (me XOR j), landing in
                # rcv[:, j] there. Receiver r's slot j is written only by
                # core (r XOR j) -> no collisions, no runtime registers.
                # Desc-gen (the 7 preps) is off the critical path; the
                # trigger carries the RAW dep on acc_col.
                rs, ls = rdma_sems
                rcv = const.tile([P, ncores], F32, name="rcv")
                nc.vector.memset(rcv[:], 0.0)
                for j in range(1, ncores):
                    rd = [None] * ncores
                    rd[j] = (0, j)
                    nc.gpsimd.remote_dma_broadcast(
                        rcv[:, ds(j, 1)], acc_col[:], rs, ls, rdests=rd
                    )
                nc.gpsimd.trigger_dma(count=None)
                nc.vector.tensor_copy(rcv[:, 0:1], acc_col[:])
                tot_col = const.tile([P, 1], F32, name="tot_col")
                red = nc.vector.tensor_reduce(
                    tot_col[:],
                    rcv[:],
                    axis=mybir.AxisListType.X,
                    op=mybir.AluOpType.add,
                )
                # The sem-ge wait for the 7 remote arrivals (16 each) is
                # injected onto `red` AFTER tile scheduling (build_module):
                # the single-core scheduling sim cannot observe remote
                # increments and would report a deadlock.
                clr = nc.gpsimd.sem_clear(rs)
                add_dep_helper(clr.ins, red.ins, sync=True)
                # global sum / T via PE cross-partition reduce (inv_col = 1/T)
                psum_g = ptp.tile([P, BANK], F32, name="tp", tag="tp")
                nc.tensor.matmul(
                    psum_g[0:1, 0:1], inv_col[:], tot_col[:], start=True, stop=True
                )
                nc.scalar.copy(gmean[:], psum_g[0:1, 0:1])
            elif coll_mode == "allgather":
                # local sum/T -> [1,1] via PE cross-partition matmul
                psum_s = ptp.tile([P, BANK], F32, name="tp", tag="tp")
                nc.tensor.matmul(
                    psum_s[0:1, 0:1], inv_col[:], acc_col[:], start=True, stop=True
                )
                lmean = const.tile([1, 1], F32, name="lmean")
                nc.scalar.copy(lmean[:], psum_s[0:1, 0:1])
                cc_in = dram.tile([1, 1], F32, name="cc_in")
                # gather along the partition dim: [ncores, 1] so the result
                # lands on ncores partitions, letting ONE k=ncores PE matmul
                # do the sum AND the 128-partition broadcast together
                cc_out = dram.tile([ncores, 1], F32, name="cc_out", addr_space="Shared")
                nc.sync.dma_start(cc_in[:], lmean[:])
                nc.gpsimd.collective_compute(
                    "AllGather",
                    mybir.AluOpType.bypass,
                    replica_groups=[list(range(ncores))],
                    ins=[cc_in[:].opt()],
                    outs=[cc_out[:].opt()],
                )
                gm8c = const.tile([ncores, 1], F32, name="gm8c")
                nc.sync.dma_start(gm8c[:], cc_out[:])
                ones8 = const.tile([ncores, P], F32, name="ones8")
                nc.vector.memset(ones8[:], 1.0)
                psum_m = pmm.tile([P, BANK], F32, name="mm", tag="mm")
                nc.tensor.matmul(
                    psum_m[:, 0:1], ones8[:], gm8c[:], start=True, stop=True
                )
            else:
                if coll_mode == "none":  # core-local mean stand-in (timing)
                    psum_s = ptp.tile([P, BANK], F32, name="tp", tag="tp")
                    nc.tensor.matmul(
                        psum_s[0:1, 0:1], inv_col[:], acc_col[:], start=True, stop=True
                    )
                    lmean = const.tile([1, 1], F32, name="lmean")
                    nc.scalar.copy(lmean[:], psum_s[0:1, 0:1])
                    nc.scalar.mul(gmean[:], lmean[:], float(ncores))
                # broadcast mean to all partitions via PE
                psum_m = pmm.tile([P, BANK], F32, name="mm", tag="mm")
                nc.tensor.matmul(
                    psum_m[:, 0:1], ones_row[:], gmean[:], start=True, stop=True
                )
            mean_col = const.tile([P, 1], F32, name="mean_col")
            nc.scalar.copy(mean_col[:], psum_m[:, 0:1])

        # bias*mean in all partitions: one vector op on the prebuilt broadcast
        bias_bb = const.tile([P, BANK], F32, name="bias_bb")
        nc.vector.tensor_scalar_mul(bias_bb[:], bias_bb_u[:], mean_col[:])

        # --- phase 3: matmul, then fused (psum*mean + bias*mean) on DVE ---
        out_dma_insts = []
        for i in range(iters):
            out_sb = outp.tile([P, T_SUB, D], F32, name="out_sb", tag="out_sb")
            for b in range(banks_per_iter):
                bank = pmm.tile([P, BANK], F32, name="mm", tag="mm")
                for t in range(t_per_bank):
                    col = i * T_SUB * P + b * BANK + t * P
                    nc.tensor.matmul(
                        bank[:, ds(t * P, P)],
                        xbT[:, ds(col, P)],
                        wsT[:],
                        start=True,
                        stop=True,
                    )
                nc.vector.scalar_tensor_tensor(
                    out_sb[:, ds(b * t_per_bank, t_per_bank), :],
                    bank[:],
                    mean_col[:],
                    bias_bb[:],
                    mybir.AluOpType.mult,
                    mybir.AluOpType.add,
                )
            out_dma_insts.append(nc.sync.dma_start(out_view[i], out_sb[:]))
    ret = {"x_loads": x_load_insts, "out_dmas": out_dma_insts}
    if coll_mode == "rdma":
        ret["rcv"] = rcv
        ret["rdma_red"] = red
    return ret


def build_module(n_per_core=N_PER_CORE, ncores=NCORES, repeats=1, coll_mode="allgather", rdma_probe_delay=0):
    nc = bacc.Bacc(
        "TRN2",
        target_bir_lowering=False,
        debug=False,
        enable_asserts=False,
        num_devices=ncores,
    )
    x_t = nc.dram_tensor("x", [n_per_core, D], F32, kind="ExternalInput")
    w_t = nc.dram_tensor("weight", [D, D], F32, kind="ExternalInput")
    b_t = nc.dram_tensor("bias", [D], F32, kind="ExternalInput")
    o_t = nc.dram_tensor("out", [n_per_core, D], F32, kind="ExternalOutput")
    import contextlib

    from concourse.tile import add_dep_helper

    rdma_sems = None
    rdma_reds = []
    if coll_mode == "rdma":
        rdma_sems = (nc.alloc_semaphore("rdma_rs"), nc.alloc_semaphore("rdma_ls"))

    with tile.TileContext(nc) as tc:
        with contextlib.ExitStack() as ctx:
            pools = make_pools(tc, ctx)
            prev_out = None
            for r in range(repeats):
                if r:
                    tc.strict_bb_all_engine_barrier()
                insts = emit(
                    tc,
                    pools,
                    o_t.ap(),
                    x_t.ap(),
                    w_t.ap(),
                    b_t.ap(),
                    total_elems=n_per_core * ncores * D,
                    ncores=ncores,
                    coll_mode=coll_mode,
                    rdma_sems=rdma_sems,
                    rdma_probe_delay=rdma_probe_delay,
                )
                if prev_out is not None:
                    # serialize repeats at the DMA level too (the barrier only
                    # gates compute): every x-load waits the previous repeat's
                    # final out-DMA, so slope timing = honest single-exec time
                    for ld in insts["x_loads"]:
                        add_dep_helper(ld.ins, prev_out.ins, sync=True)
                prev_out = insts["out_dmas"][-1]
                if coll_mode == "rdma":
                    rdma_reds.append(insts["rdma_red"].ins)

    if coll_mode == "rdma":
        # Inject the immediate sem-ge wait for the 7 remote arrivals (16
        # per arrival) onto each repeat's rcv-reduce, post-scheduling: the
        # single-core scheduling sim cannot see remote increments and would
        # deadlock on this wait if it were visible to it. rs is cleared at
        # the end of each repeat, so the threshold is constant per repeat.
        rs = rdma_sems[0]
        for red_ins in rdma_reds:
            si = red_ins.sync_info
            w = mybir.SyncWait(
                sync_type="semaphore",
                id=rs.num,
                wait_mode="sem-ge-imm",
                wait_value=2 * (ncores - 1),
                ant_name="rdma_rs",
            )
            ow = list(si.on_wait) if si is not None else []
            ow.append(w)
            ou = list(si.on_update) if si is not None else []
            red_ins.sync_info = mybir.SyncInfo(on_wait=ow, on_update=ou)
    nc.compile()
    return nc


_CACHE = {}


def get_module(n_per_core=N_PER_CORE, ncores=NCORES, repeats=1, coll_mode="allgather"):
    key = (n_per_core, ncores, repeats, coll_mode)
    if key not in _CACHE:
        _CACHE[key] = build_module(n_per_core, ncores, repeats, coll_mode=coll_mode)
    return _CACHE[key]


def kernel(x, weight, bias):
    x = np.ascontiguousarray(np.asarray(x, dtype=np.float32))
    weight = np.ascontiguousarray(np.asarray(weight, dtype=np.float32))
    bias = np.ascontiguousarray(np.asarray(bias, dtype=np.float32))
    assert x.shape == (N_TOTAL, D), x.shape

    nc = get_module()
    in_maps = [
        {
            "x": x[c * N_PER_CORE : (c + 1) * N_PER_CORE],
            "weight": weight,
            "bias": bias,
        }
        for c in range(NCORES)
    ]
    res = bass_utils.run_bass_kernel_spmd(nc, in_maps, core_ids=list(range(NCORES)))
    return np.concatenate([r["out"] for r in res.results], axis=0)


if __name__ == "__main__":
    import time

    t0 = time.time()
    nc = build_module()
    print("build+compile OK in", time.time() - t0, "s")

